# revision 6
# baseline (speedup 1.0000x reference)
"""Cross-image contrastive loss on 8 TRN2 NeuronCores.

Strategy (row-parallel over the N=4096 pixel dim, 512 rows per core):
  - Host precomputes all O(N) prep: label-count weights w, diag terms t1,
    one-hot mask folding for S2 (augmented K = d + L + 1 = 84 contraction).
  - Device does the O(N^2) work: S1 = Fi^T Fii and S2 (mask folded) as
    16 PSUM groups of [128, 2048] raw logits.
  - exp + row-sum is split across TWO engines: the Act engine does exact
    Exp(scale=1/tau) with accum_out; the Vector engine (DVE) approximates
    exp via the Schraudolph bit trick (s = l*A1 + B1 -> int16 -> bitcast
    as bf16 is 2^((s-16256)/128) with linear mantissa interp), then a 4x
    tensor_scalar pass accumulates the bf16 dump.
  - Z per pixel = sum of its 4 group accums; loss = sum w*(t1 - 2*ln Z).
  - Each core emits its partial loss; host sums the 8 partials.
"""

import math
import sys

import numpy as np

sys.path.insert(0, "/opt/trn_rl_repo")

import ml_dtypes

TAU = 0.07
EPS = 1e-4
L = 19
D = 64
N = 4096
NCORES = 8
P = N // NCORES  # 512 rows per core
KA = D + L + 1  # 84 augmented contraction for S2
CMASK = 4.25  # bf16-exact mask magnitude; CMASK/TAU ~ 60.7 in the exponent
PB = P // 128  # 4 partition blocks per core

# Schraudolph constants: bf16 bits v ~ round(l*A1 + B1) give
# 2^((v-16256)/128) ~ exp(l/TAU).  B1 centered so E[approx/exact] ~ 1.
A1 = 128.0 / (TAU * math.log(2.0))
B1 = 16256.0 - 7.37

# consumer pattern per group: 'A' = Act engine exact exp, 'D' = DVE approx
PATTERN = "ADADADADADADADAA"

_compiled = None


def _build():
    from concourse import bacc, mybir, tile

    f32 = mybir.dt.float32
    bf16 = mybir.dt.bfloat16
    i16 = mybir.dt.int16
    Exp = mybir.ActivationFunctionType.Exp
    Ln = mybir.ActivationFunctionType.Ln
    X = mybir.AxisListType.X
    add = mybir.AluOpType.add
    mult = mybir.AluOpType.mult

    nc = bacc.Bacc("TRN2", target_bir_lowering=False, debug=False)

    lhs1_d = nc.dram_tensor("lhs1", (D, P), bf16, kind="ExternalInput")
    lhs2_d = nc.dram_tensor("lhs2", (KA, P), bf16, kind="ExternalInput")
    rhs1_d = nc.dram_tensor("rhs1", (D, N), bf16, kind="ExternalInput")
    rhs2_d = nc.dram_tensor("rhs2", (KA, N), bf16, kind="ExternalInput")
    wt_d = nc.dram_tensor("wt", (128, 2 * PB), f32, kind="ExternalInput")
    out_d = nc.dram_tensor("out", (1, 1), f32, kind="ExternalOutput")

    with tile.TileContext(nc) as tc:
        with (
            tc.tile_pool(name="res", bufs=1) as res,
            tc.tile_pool(name="dmp", bufs=2) as dmp,
            tc.tile_pool(name="ps", bufs=2, space="PSUM") as psp,
        ):
            # preload the act table set that serves BOTH Exp and Ln so the
            # pass doesn't emit a second mid-kernel ACT_TABLE_LOAD
            nc.scalar.add_instruction(
                mybir.InstLoadActFuncSet(
                    name=nc.get_next_instruction_name(),
                    act_func_set_id=6,  # natural_log_exp_and_others
                    ins=[],
                    outs=[],
                )
            )

            # ---- resident SBUF tensors ----
            lhs1_sb = res.tile([D, P], bf16, tag="lhs1")
            lhs2_sb = res.tile([KA, P], bf16, tag="lhs2")
            rhs1_sb = res.tile([D, N], bf16, tag="rhs1")
            rhs2_sb = res.tile([KA, N], bf16, tag="rhs2")
            wt_sb = res.tile([128, 2 * PB], f32, tag="wt")
            acc = res.tile([128, 16], f32, tag="acc")
            zeros = res.tile([128, 1], f32, tag="zeros")
            ones = res.tile([128, 1], f32, tag="ones")
            adump = res.tile([128, 2048], bf16, tag="adump")
            pdummy = res.tile([128, 2048], bf16, tag="pdummy")
            nc.vector.memset(zeros[:], 0.0)
            nc.vector.memset(ones[:], 1.0)

            # DMA order = first-use order: lhs1 + rhs1 chunks feed the
            # S1 groups, then lhs2/rhs2 for S2, then the tiny wt tile.
            nc.sync.dma_start(lhs1_sb[:], lhs1_d[:])
            for c in range(4):
                cs = slice(c * 1024, (c + 1) * 1024)
                nc.sync.dma_start(rhs1_sb[:, cs], rhs1_d[:, cs])
            nc.sync.dma_start(lhs2_sb[:], lhs2_d[:])
            for c in range(4):
                cs = slice(c * 1024, (c + 1) * 1024)
                nc.sync.dma_start(rhs2_sb[:, cs], rhs2_d[:, cs])
            nc.sync.dma_start(wt_sb[:], wt_d[:])

            # ---- 16 groups: matmul -> exp+row-sum ----
            # group g = (s, b, h): s in {0: S1, 1: S2}, p-block b, col-half h
            for g in range(16):
                s, b, h = g >> 3, (g >> 1) & 3, g & 1
                col = b * 4 + s * 2 + h  # acc col: p-block-major
                lhs_sb = lhs1_sb if s == 0 else lhs2_sb
                rhs_sb = rhs1_sb if s == 0 else rhs2_sb
                ps = psp.tile([128, 2048], f32, tag="mm")
                for j in range(4):
                    cs = slice(h * 2048 + j * 512, h * 2048 + (j + 1) * 512)
                    nc.tensor.matmul(
                        ps[:, j * 512 : (j + 1) * 512],
                        lhs_sb[:, b * 128 : (b + 1) * 128],
                        rhs_sb[:, cs],
                        start=True,
                        stop=True,
                    )
                if PATTERN[g] == "A":
                    nc.scalar.activation(
                        adump[:],
                        ps[:],
                        Exp,
                        bias=zeros[:],
                        scale=1.0 / TAU,
                        accum_out=acc[:, col : col + 1],
                    )
                else:
                    ddump = dmp.tile([128, 2048], i16, tag="ddump")
                    nc.vector.tensor_scalar(ddump[:], ps[:], A1, B1, mult, add)
                    nc.vector.tensor_scalar(
                        pdummy[:],
                        ddump[:].bitcast(bf16),
                        1.0,
                        None,
                        mult,
                        add,
                        accum_out=acc[:, col : col + 1],
                    )

            # ---- Z = sum of the 4 group-sums per p-block, then logZ ----
            zpm = res.tile([128, PB], f32, tag="zpm")
            nc.vector.tensor_reduce(
                zpm[:],
                acc[:].rearrange("p (b q) -> p b q", q=4),
                axis=X,
                op=add,
            )
            nc.vector.tensor_scalar_add(zpm[:], zpm[:], EPS)
            logz = res.tile([128, PB], f32, tag="logz")
            nc.scalar.activation(logz[:], zpm[:], Ln, bias=zeros[:])

            # ---- values = w * (t1 - 2*logZ); partial = sum ----
            vals = res.tile([128, PB], f32, tag="vals")
            nc.vector.scalar_tensor_tensor(
                out=vals[:],
                in0=logz[:],
                scalar=-2.0,
                in1=wt_sb[:, PB : 2 * PB],
                op0=mult,
                op1=add,
            )
            nc.vector.tensor_mul(vals[:], vals[:], wt_sb[:, 0:PB])
            vred = res.tile([128, 1], f32, tag="vred")
            nc.vector.tensor_reduce(vred[:], vals[:], axis=X, op=add)

            fin = psp.tile([128, 2048], f32, tag="mm")
            nc.tensor.matmul(
                fin[0:1, 0:1], ones[:], vred[:], start=True, stop=True
            )
            res_sb = res.tile([1, 1], f32, tag="res")
            nc.scalar.copy(res_sb[:], fin[0:1, 0:1])
            nc.sync.dma_start(out_d[:], res_sb[:])

    nc.compile()
    return nc


def _make_in_maps(features_i, features_ii, features_jj, i, ii, jj):
    bf16 = ml_dtypes.bfloat16
    Fi = features_i.reshape(D, N).astype(np.float32)
    Fii = features_ii.reshape(D, N).astype(np.float32)
    Fjj = features_jj.reshape(D, N).astype(np.float32)
    lab = i.reshape(-1)
    ii_f = ii.reshape(-1)
    jj_f = jj.reshape(-1)

    lids = np.arange(L, dtype=np.int32)
    oh_jj = (jj_f[None, :] == lids[:, None]).astype(np.float32)  # [L, N]

    rhs1 = Fii.astype(bf16)
    rhs2 = np.zeros((KA, N), np.float32)
    rhs2[0:D] = Fjj
    rhs2[D : D + L] = CMASK * oh_jj
    rhs2[D + L] = -CMASK
    rhs2 = rhs2.astype(bf16)

    cnt_ii = np.bincount(ii_f, minlength=L).astype(np.float32)
    cnt_jj = np.bincount(jj_f, minlength=L).astype(np.float32)
    wl = cnt_ii / (cnt_ii + cnt_jj + EPS)

    in_maps = []
    for c in range(NCORES):
        sel = slice(c * P, (c + 1) * P)
        lab_c = lab[sel]
        Fic = Fi[:, sel]

        lhs2 = np.zeros((KA, P), np.float32)
        lhs2[0:D] = Fic
        lhs2[D : D + L] = (lab_c[None, :] == lids[:, None]).astype(np.float32)
        lhs2[D + L] = 1.0

        w = -wl[lab_c] / N  # [P]
        t1 = (Fic * (Fii[:, sel] + Fjj[:, sel])).sum(0) / TAU  # [P]
        wt = np.zeros((128, 2 * PB), np.float32)
        wt[:, 0:PB] = w.reshape(PB, 128).T
        wt[:, PB : 2 * PB] = t1.reshape(PB, 128).T

        in_maps.append(
            {
                "lhs1": Fic.astype(bf16),
                "lhs2": lhs2.astype(bf16),
                "rhs1": rhs1,
                "rhs2": rhs2,
                "wt": wt,
            }
        )
    return in_maps


_LDW_PATCHED = False


def _enable_ldw_opt():
    """Flip walrus --enable-ldw-opt for this process (dedups back-to-back
    LDWEIGHTS of the same stationary operand)."""
    global _LDW_PATCHED
    if _LDW_PATCHED:
        return
    from concourse import bass_utils

    orig = bass_utils.run_command

    def patched(cmd, *a, **kw):
        if isinstance(cmd, list):
            cmd = [
                "--enable-ldw-opt=true" if c == "--enable-ldw-opt=false" else c
                for c in cmd
            ]
        return orig(cmd, *a, **kw)

    bass_utils.run_command = patched
    _LDW_PATCHED = True


def kernel(features_i, features_ii, features_jj, i, ii, jj):
    global _compiled
    from concourse import bass_utils

    if _compiled is None:
        _compiled = _build()
    in_maps = _make_in_maps(features_i, features_ii, features_jj, i, ii, jj)
    results = bass_utils.run_bass_kernel_spmd(
        _compiled, in_maps, core_ids=list(range(NCORES))
    )
    total = np.float32(0.0)
    for r in results.results:
        total += np.float32(r["out"].reshape(-1)[0])
    return np.array(total, dtype=np.float32)
